# revision 37
# baseline (speedup 1.0000x reference)
"""ChebConv (complex, K+1=3 hops) Trainium2 kernel over 8 NeuronCores, v4.

Sharding: 1D node partition on destination rows (6250 rows/core), each core
processes exactly the edges targeting its rows.

v4 vs v3:
- two-tier group packing: rows are packed into groups whose edge sums sit
  just under 5*128 or 6*128, minimizing sum(ceil(deg_g/128)) -> ~12% fewer
  128-edge blocks (less HBM G-stream traffic, fewer stage-1 matmuls and
  smaller V build). Groups sorted by block count desc so the cross-core max
  profile stays tight.
- V build merged: one is_equal and one mult per BATCH (36 blocks) instead
  of two DVE ops per block, via 3D/4D broadcast APs.
- outputs written bf16 (halves output DMA traffic; well within tolerance).
"""
import sys
sys.path.insert(0, '/opt/trn_rl_repo')

import numpy as np
import ml_dtypes

N = 50000
E = 1_600_000
K1 = 3
C = 256
CORES = 8
RPC = N // CORES            # 6250 rows per core
GR = 21                     # rows per group
MCOLS = 6 * GR              # 126 one-hot columns
GPB = 6                     # groups per batch
ROWS_PB = GR * GPB          # 126
REAL_GRP = -(-RPC // GR)    # 298
NB = -(-REAL_GRP // GPB)    # 50
NGRP = NB * GPB             # 300
LCAP = 5 * 128 - 4          # light group edge cap (5 blocks)
HCAP = 6 * 128 - 4          # heavy group edge cap (6 blocks)


def _bf16(x):
    return x.astype(ml_dtypes.bfloat16)


def _lpt(row_ids, d, nbins):
    """LPT row_ids (by degree desc) into nbins bins of <=GR rows.
    Returns (members, sums)."""
    order = row_ids[np.argsort(-d[row_ids], kind="stable")]
    sums = np.zeros(nbins, np.int64)
    members = [[] for _ in range(nbins)]
    full_pen = np.zeros(nbins)
    for r in order:
        g = int(np.argmin(sums + full_pen))
        members[g].append(int(r))
        sums[g] += d[r]
        if len(members[g]) >= GR:
            full_pen[g] = np.inf
    return members, sums


def _pack_core(d):
    """Pack one core's rows into REAL_GRP groups of <=GR rows with group
    edge sums packed into two tiers (just under 5*128 / 6*128) so that
    sum(ceil(sum_g/128)) is near-minimal: choose a heavy-tier multiset of
    GR*k rows (top t + bottom b of the degree-sorted rows, tuned to hit
    k*HTGT total), LPT each tier separately, sort groups by block count.

    Returns slot_of_row [RPC] and row_of_slot [REAL_GRP*GR].
    """
    LTGT, HTGT = 635, 758
    tot = int(d.sum())
    k = max(1, min(REAL_GRP - 1, -(-(tot - REAL_GRP * LTGT) // (HTGT - LTGT))))

    order = np.argsort(-d, kind="stable")
    ds = d[order].astype(np.int64)
    P = np.concatenate([[0], np.cumsum(ds)])
    nh = GR * k
    # heavy multiset = top t + bottom (nh-t); pick t so the sum ~= k*HTGT
    ts = np.arange(0, nh + 1)
    hsum = P[ts] + (P[RPC] - P[RPC - (nh - ts)])
    t = int(ts[np.argmin(np.abs(hsum - k * HTGT))])
    b = nh - t
    heavy_ids = np.concatenate([order[:t], order[RPC - b:]]) if b else order[:t]
    light_ids = order[t:RPC - b] if b else order[t:]

    mh, sh = _lpt(heavy_ids, d, k)
    ml, sl = _lpt(light_ids, d, REAL_GRP - k)
    members = mh + ml
    sums = np.concatenate([sh, sl])

    # sort groups by actual block count desc (then sum desc)
    nblk = -(-sums // 128)
    perm = sorted(range(REAL_GRP), key=lambda g: (-nblk[g], -sums[g]))
    slot_of_row = np.empty(RPC, np.int64)
    row_of_slot = np.full(REAL_GRP * GR, -1, np.int64)
    for newg, g in enumerate(perm):
        for j, r in enumerate(members[g]):
            slot_of_row[r] = newg * GR + j
            row_of_slot[newg * GR + j] = r
    return slot_of_row, row_of_slot


def _preprocess(rows, cols, Lr, Li, weight, bias):
    rows = np.asarray(rows).astype(np.int64)
    cols = np.asarray(cols).astype(np.int64)
    core = rows // RPC
    rloc = rows - core * RPC

    degs = [np.bincount(rloc[core == c], minlength=RPC) for c in range(CORES)]
    assigns = [_pack_core(d) for d in degs]
    slot_of_row = np.stack([a[0] for a in assigns])     # [CORES, RPC]
    row_of_slot = np.stack([a[1] for a in assigns])     # [CORES, REAL_GRP*GR]

    slot = slot_of_row[core, rloc]                      # [E]
    g = slot // GR
    jl = (slot - g * GR).astype(np.float32)

    C6 = np.empty((E, 6), np.float32)
    C6[:, 0:3] = np.asarray(Lr).T
    C6[:, 3:6] = np.asarray(Li).T

    key = core * NGRP + g
    order = np.lexsort((cols, key))
    key_s = key[order]
    nbuck = CORES * NGRP
    bounds = np.searchsorted(key_s, np.arange(nbuck + 1))
    cnt = (bounds[1:] - bounds[:-1]).reshape(CORES, NGRP)

    ne_max = cnt.max(axis=0)                            # [NGRP]
    nblk_g = -(-ne_max // 128)                          # [NGRP]
    nblk_g[:REAL_GRP] = np.maximum(nblk_g[:REAL_GRP], 1)
    bs0 = np.concatenate([[0], np.cumsum(nblk_g)])
    tot_blk = int(bs0[-1])
    nbt_max = int(max(bs0[(bt + 1) * GPB] - bs0[bt * GPB] for bt in range(NB)))

    per_core = []
    cols_s = cols[order]
    C6_s = C6[order]
    jl_s = jl[order]
    for c in range(CORES):
        idxg = np.full(tot_blk * 128, N, np.int64)      # N -> zero row
        c6t = np.zeros((128, tot_blk * 6), np.float32)
        jlf = np.zeros((128, tot_blk), np.float32)
        for gi in range(NGRP):
            buck = c * NGRP + gi
            lo, hi = bounds[buck], bounds[buck + 1]
            ne = hi - lo
            if ne == 0:
                continue
            bs = bs0[gi]
            idxg[bs * 128: bs * 128 + ne] = cols_s[lo:hi]
            cc = C6_s[lo:hi]
            jj = jl_s[lo:hi]
            nb = int(nblk_g[gi])
            for k in range(nb):
                a, b = k * 128, min((k + 1) * 128, ne)
                if a >= b:
                    break
                c6t[0:b - a, (bs + k) * 6:(bs + k) * 6 + 6] = cc[a:b]
                jlf[0:b - a, bs + k] = jj[a:b]
        per_core.append(dict(
            idxg=idxg,
            c6=np.ascontiguousarray(_bf16(c6t)),
            jl=np.ascontiguousarray(_bf16(jlf)),
        ))

    # weight tiles [6][128, 256] bf16: fh*3+k = W[k][fh] (signs are folded
    # into the D/S plane combines)
    weight = np.asarray(weight, np.float32)
    wt = np.empty((6, 128, C), np.float32)
    for fh in range(2):
        for k in range(K1):
            wt[fh * 3 + k] = weight[k][fh * 128:(fh + 1) * 128]
    wsb = np.ascontiguousarray(_bf16(wt.transpose(1, 0, 2).reshape(128, 6 * C)))

    # bias as a stage-2 outer-product operand: row 0 = bias, rest 0;
    # paired with a ones-row lhsT it adds bias[c] to every output row
    biasr = np.zeros((128, C), np.float32)
    biasr[0] = np.asarray(bias, np.float32)[0]
    biasr = np.ascontiguousarray(_bf16(biasr))
    ones1 = np.zeros((128, MCOLS), np.float32)
    ones1[0] = 1.0
    ones1 = np.ascontiguousarray(_bf16(ones1))
    # V column m = j*6 + s  ->  j = m // 6 ; tiled per block across a batch
    md1 = (np.arange(MCOLS) // 6).astype(np.float32)
    mdbig = np.ascontiguousarray(
        _bf16(np.tile(md1, (128, nbt_max))))

    return dict(nblk_g=nblk_g, bs0=bs0, tot_blk=tot_blk, nbt_max=nbt_max,
                per_core=per_core, wsb=wsb, biasr=biasr, ones1=ones1,
                mdbig=mdbig, row_of_slot=row_of_slot)


def _final_mm_list():
    """(target, fh, k): pbuf plane (tgt*2+fh)*3+k @ W tile fh*3+k.

    Plane combines (done on DVE during PSUM->SBUF):
      D[fh,k] = P(Xr half fh, s=k)   - P(Xi half fh, s=3+k)   -> real
      S[fh,k] = P(Xr half fh, s=3+k) + P(Xi half fh, s=k)     -> imag
    """
    return [(tgt, fh, k)
            for tgt in range(2) for fh in range(2) for k in range(K1)]


def _build(nc, prep, repeat=1):
    import os
    import concourse.mybir as mybir
    from concourse.tile import TileContext
    import contextlib

    abl = os.environ.get("CHEB_ABL", "full")  # dma | s1 | s1t | full
    f32 = mybir.dt.float32
    bf16 = mybir.dt.bfloat16
    tot_blk = prep["tot_blk"]
    nblk_g = prep["nblk_g"]
    bs0 = prep["bs0"]
    nbt_max = prep["nbt_max"]

    gstr_d = nc.dram_tensor("gstr", [128, tot_blk * 512], bf16,
                            kind="ExternalInput")
    c6_d = nc.dram_tensor("c6", [128, tot_blk * 6], bf16, kind="ExternalInput")
    jl_d = nc.dram_tensor("jl", [128, tot_blk], bf16, kind="ExternalInput")
    w_d = nc.dram_tensor("wt", [128, 6 * C], bf16, kind="ExternalInput")
    bias_d = nc.dram_tensor("biasr", [128, C], bf16, kind="ExternalInput")
    on_d = nc.dram_tensor("ones1", [128, MCOLS], bf16, kind="ExternalInput")
    md_d = nc.dram_tensor("mdbig", [128, nbt_max * MCOLS], bf16,
                          kind="ExternalInput")
    o_d = nc.dram_tensor("out_ri", [NB * ROWS_PB, 2 * C], bf16,
                         kind="ExternalOutput")

    mms = _final_mm_list()

    with TileContext(nc) as tc:
        with tc.tile_pool(name="const", bufs=1) as cpool, \
             tc.tile_pool(name="g", bufs=3) as gpool, \
             tc.tile_pool(name="v", bufs=3) as vpool, \
             tc.tile_pool(name="ptb", bufs=3) as ptbpool, \
             tc.tile_pool(name="pb", bufs=2) as pbpool, \
             tc.tile_pool(name="os", bufs=4) as ospool, \
             tc.tile_pool(name="pt", bufs=3, space="PSUM") as ptpool, \
             tc.tile_pool(name="po", bufs=2, space="PSUM") as popool:

            c6_t = cpool.tile([128, tot_blk * 6], bf16)
            jl_t = cpool.tile([128, tot_blk], bf16)
            w_t = cpool.tile([128, 6 * C], bf16)
            bias_t = cpool.tile([128, C], bf16)
            on_t = cpool.tile([128, MCOLS], bf16)
            md_t = cpool.tile([128, nbt_max * MCOLS], bf16)
            for dst, src in [(c6_t, c6_d), (jl_t, jl_d),
                             (w_t, w_d), (bias_t, bias_d), (on_t, on_d),
                             (md_t, md_d)]:
                nc.sync.dma_start(dst[:], src[:])

            def emit_stage2(pbuf, bt):
                # final matmuls for batch bt (bf16): P^T planes @ W
                po_r = popool.tile([128, C], f32, tag="por")
                po_i = popool.tile([128, C], f32, tag="poi")
                nmm = {0: 0, 1: 0}
                for tgt, fh, k in mms:
                    po = po_r if tgt == 0 else po_i
                    plane = (tgt * 2 + fh) * 3 + k
                    wi = fh * 3 + k
                    lhsT = pbuf[:, plane * MCOLS:(plane + 1) * MCOLS]
                    nc.tensor.matmul(
                        po[:MCOLS, :], lhsT, w_t[:, wi * C:(wi + 1) * C],
                        start=(nmm[tgt] == 0), stop=False)
                    nmm[tgt] += 1
                # 13th matmul per target: ones-row x bias-row adds bias[c]
                # to every output row (keeps DVE free of the bias add)
                for po in (po_r, po_i):
                    nc.tensor.matmul(po[:MCOLS, :], on_t[:], bias_t[:],
                                     start=False, stop=True)
                o_t = ospool.tile([128, 2 * C], bf16, tag="o")
                # out-DMA on Pool SWDGE: keeps the SP queue free for gt
                # prefetch (engine DMA queues execute in order; a blocked
                # out-DMA on sync would stall the next batch's gt load).
                # PSUM->SBUF moves on ACT so the DVE queue only carries
                # V builds (an in-order DVE bias-add here would delay the
                # next batch's V and stall PE).
                nc.scalar.copy(o_t[:MCOLS, :C], po_r[:MCOLS, :])
                nc.scalar.copy(o_t[:MCOLS, C:], po_i[:MCOLS, :])
                nc.gpsimd.dma_start(o_d[bt * ROWS_PB:(bt + 1) * ROWS_PB, :],
                                    o_t[:MCOLS, :])

            rep_cm = tc.For_i(0, repeat, 1) if repeat > 1 else contextlib.nullcontext()
            with rep_cm:
              pending = None
              for bt in range(NB):
                  b_lo = int(bs0[bt * GPB])
                  b_hi = int(bs0[(bt + 1) * GPB])
                  nbt = b_hi - b_lo
                  gt = gpool.tile([128, nbt_max * 512], bf16, tag="g")
                  nc.sync.dma_start(gt[:, :nbt * 512],
                                    gstr_d[:, b_lo * 512:b_hi * 512])
                  if abl == "dma":
                      continue
                  # V for the whole batch: one-hot(jl) * c6, 2 DVE ops
                  v_t = vpool.tile([128, nbt_max * MCOLS], bf16, tag="v")
                  nc.vector.tensor_tensor(
                      v_t[:, :nbt * MCOLS].rearrange(
                          "p (b m) -> p b m", m=MCOLS),
                      md_t[:, :nbt * MCOLS].rearrange(
                          "p (b m) -> p b m", m=MCOLS),
                      jl_t[:, b_lo:b_hi].unsqueeze(2)
                          .broadcast_to((128, nbt, MCOLS)),
                      mybir.AluOpType.is_equal)
                  nc.vector.tensor_tensor(
                      v_t[:, :nbt * MCOLS].rearrange(
                          "p (b x s) -> p b x s", x=GR, s=6),
                      v_t[:, :nbt * MCOLS].rearrange(
                          "p (b x s) -> p b x s", x=GR, s=6),
                      c6_t[:, b_lo * 6:b_hi * 6].rearrange(
                          "p (b s) -> p b s", s=6).unsqueeze(2)
                          .broadcast_to((128, nbt, GR, 6)),
                      mybir.AluOpType.mult)
                  pbuf = pbpool.tile([128, 12 * ROWS_PB], bf16, tag="pbuf")
                  for gl in range(GPB):
                      gi = bt * GPB + gl
                      nb_tot = int(nblk_g[gi])
                      if nb_tot == 0:
                          nc.vector.memset(
                              pbuf[:].rearrange(
                                  "p (pi g j) -> p pi g j", pi=12, g=GPB)[
                                  :, :, gl, :], 0.0)
                          continue
                      gbs = int(bs0[gi])
                      # stage-1, operand-swapped: lhsT = G feature-quadrant
                      # (stationary), rhs = V (moving). PSUM accumulates
                      # P^T[feat_q, (j,s)] directly -- no transposes needed.
                      p_t = ptpool.tile([128, 512], f32, tag="pt")
                      for b in range(nb_tot):
                          lb = gbs + b - b_lo
                          for q in range(4):
                              # one accumulation group for the whole bank:
                              # start zeroes the entire PSUM bank, so only
                              # the first matmul may set it
                              nc.tensor.matmul(
                                  p_t[:, q * MCOLS:(q + 1) * MCOLS],
                                  gt[:, lb * 512 + q * 128:
                                      lb * 512 + (q + 1) * 128],
                                  v_t[:, lb * MCOLS:(lb + 1) * MCOLS],
                                  start=(b == 0 and q == 0),
                                  stop=(b == nb_tot - 1 and q == 3))
                      if abl == "s1":
                          continue
                      # combine plane pairs sharing a W tile while moving
                      # PSUM->SBUF (DVE): p_t cols = q*126 + (j*6+s) with
                      # q = xpart*2+fh; for each (tgt, fh) write 3 k-planes
                      #   D[fh,k] = Pr(s=k) - Pi(s=3+k)   (tgt 0, real)
                      #   S[fh,k] = Pr(s=3+k) + Pi(s=k)   (tgt 1, imag)
                      ptb = ptbpool.tile([128, 4 * MCOLS], bf16, tag="ptb")
                      nc.scalar.copy(ptb[:], p_t[:, :4 * MCOLS])
                      pt_v = ptb[:].rearrange(
                          "p (q j s) -> p q s j", q=4, s=6)
                      for tgt in range(2):
                          for fh in range(2):
                              w_ = tgt * 2 + fh
                              dst = pbuf[:, w_ * 3 * ROWS_PB:
                                         (w_ + 1) * 3 * ROWS_PB].rearrange(
                                  "p (k g j) -> p k g j", k=3, g=GPB)[
                                  :, :, gl, :]
                              a = pt_v[:, fh, 3 * tgt:3 * tgt + 3, :]
                              b = pt_v[:, 2 + fh, 3 - 3 * tgt:6 - 3 * tgt, :]
                              nc.vector.tensor_tensor(
                                  dst, a, b,
                                  mybir.AluOpType.subtract if tgt == 0
                                  else mybir.AluOpType.add)
                      if gl == 1 and pending is not None and abl == "full":
                          emit_stage2(*pending)
                          pending = None
                  if abl in ("s1", "s1t"):
                      continue
                  if pending is not None:
                      emit_stage2(*pending)
                  pending = (pbuf, bt)
              if pending is not None and abl == "full":
                  emit_stage2(*pending)
                  pending = None


def _make_nc(prep, repeat=1):
    import concourse.bacc as bacc
    nc = bacc.Bacc("TRN2", target_bir_lowering=False, debug=False)
    _build(nc, prep, repeat=repeat)
    nc.compile()
    return nc


def _in_maps(prep, X_real, X_imag):
    xcat = _bf16(np.concatenate(
        [np.asarray(X_real, np.float32), np.asarray(X_imag, np.float32)],
        axis=1))
    xcatz = np.concatenate([xcat, np.zeros((1, 512), xcat.dtype)], axis=0)
    tot_blk = prep["tot_blk"]
    maps = []
    for c in range(CORES):
        pc = prep["per_core"][c]
        gstr = np.ascontiguousarray(
            xcatz[pc["idxg"]].reshape(tot_blk, 128, 512)
            .transpose(1, 0, 2).reshape(128, tot_blk * 512))
        maps.append({
            "gstr": gstr, "c6": pc["c6"], "jl": pc["jl"],
            "wt": prep["wsb"], "biasr": prep["biasr"],
            "ones1": prep["ones1"], "mdbig": prep["mdbig"],
        })
    return maps


def _unpermute(prep, res):
    """res: list of per-core dicts -> full [N, C] outputs."""
    out_r = np.empty((N, C), np.float32)
    out_i = np.empty((N, C), np.float32)
    nslot = REAL_GRP * GR
    for c in range(CORES):
        ros = prep["row_of_slot"][c]
        valid = ros >= 0
        rglob = c * RPC + ros[valid]
        o = res[c]["out_ri"][:nslot].astype(np.float32)
        out_r[rglob] = o[valid, :C]
        out_i[rglob] = o[valid, C:]
    return out_r, out_i


def kernel(X_real, X_imag, L_real_vals, L_imag_vals, weight, bias, rows, cols):
    from concourse.bass_utils import run_bass_kernel_spmd

    prep = _preprocess(rows, cols, L_real_vals, L_imag_vals, weight, bias)
    nc = _make_nc(prep)
    res = run_bass_kernel_spmd(nc, _in_maps(prep, X_real, X_imag),
                               core_ids=list(range(CORES)))
    return _unpermute(prep, res.results)
